# revision 15
# baseline (speedup 1.0000x reference)
"""Trainium2 Bass kernel for nn_ContinuousActor (GNN message passing actor MLP).

Strategy (pure data parallel over 8 cores, batch dim sharded, feature-major):
  - Host repacks per-pair inputs: pack(i,j) = [body(10); ones(1); A_i(24);
    A_j(24)] (K=59) where A_o = [ag_o(3); g_o(3); onehot_o(3); obj_o(15)].
    All 6 pairs share ONE stationary phi1 weight block [59, 256] (bias via
    the ones row, one-hots as data): the pair permutation becomes pure host
    data movement and phi1 needs no per-pair weights.
  - Two pairs ("duo") sit at SBUF partitions 0..58 / 64..122 and run as
    CONCURRENT matmuls via tile_position (0,0)/(64,0): phi1 costs ~half.
  - Batch tile 1024 (matmuls stay N=512 per PSUM bank): all PSUM-evacuation
    ops run at free-dim 1024 to amortize the fixed per-op engine overhead.
  - phi2 relu+sum-pool fused into DVE scalar_tensor_tensor ops:
      acc = (ph2 max -b2) add acc      (= relu(ph2+b2) - b2, accumulated)
    The constant -n_shift*b2 is folded into the rho bias host-side. The
    other half of the pairs use ACT relu (+b2 bias) with GPSIMD adds.
  - Head bias + clip run on host (device clips against bias-shifted bounds);
    saves the bias matmul and keeps the device output path to one DVE op.
  - ~72 junk warm-up matmuls at program start (overlapping the input DMA
    preamble) push the PE HAM clock gate to 8/8 before real work arrives.
"""

import numpy as np
import ml_dtypes
from contextlib import ExitStack

import concourse.bass as bass
import concourse.mybir as mybir
import concourse.tile as tile
from concourse import bacc
from concourse.bass_utils import run_bass_kernel_spmd

F32 = mybir.dt.float32
BF16 = mybir.dt.bfloat16
RELU = mybir.ActivationFunctionType.Relu
NPBF16 = ml_dtypes.bfloat16

B_FULL = 65536
N_CORES = 8
BC = B_FULL // N_CORES  # 8192 batch rows per core
BT = 1024               # batch tile (2 x 512-wide matmul free dim)
KP = 59                 # packed per-pair feature rows
PERMS = [(0, 1), (0, 2), (1, 0), (1, 2), (2, 0), (2, 1)]
LOG_SIG_MIN, LOG_SIG_MAX = -20.0, 2.0
N_WARMUP_MM = 44

# --- engine routing (baked into build AND the rho bias correction) ---
# phi2 evacuation per (mh, pair): 'stt' = DVE fused max/add (shifted by -b2,
# corrected in rho bias), 'act' = ACT relu+bias (true value; non-initial
# pairs need a GPSIMD add).
PH2_ROUTE = [['stt'] * 6, ['act'] * 6]
# h1 evacuation engines per (duo, half): each duo-mh evacuates pair A and
# pair B as two parallel FD-1024 ops on opposite engines.
H1_ENG = [('act', 'dve'), ('dve', 'act')] * 3  # indexed by duo, then (A,B)

_CACHE = {}


def _pack_256(w):
    """[256, 256] -> [128, 512] with col block (2k+m) = w[k*128:, m*128:]."""
    out = np.empty((128, 512), dtype=np.float32)
    for k in range(2):
        for m in range(2):
            out[:, (2 * k + m) * 128:(2 * k + m + 1) * 128] = \
                w[k * 128:(k + 1) * 128, m * 128:(m + 1) * 128]
    return out


def _pack_weights(phi_w1, phi_b1, phi_w2, phi_b2, rho_w1, rho_b1,
                  mean_w, mean_b, logstd_w, logstd_b):
    f = np.float32
    W1 = np.asarray(phi_w1, f)
    blk = np.concatenate([
        W1[12:22],                          # body
        np.asarray(phi_b1, f)[None, :],     # bias via ones row
        W1[0:3], W1[6:9], W1[22:25], W1[25:40],    # A_i: ag, g, onehot, feats
        W1[3:6], W1[9:12], W1[40:43], W1[43:58],   # A_j
    ], axis=0)                              # [59, 256]
    w1 = np.zeros((128, 256), dtype=f)
    w1[0:KP] = blk
    w1[64:64 + KP] = blk

    w2 = _pack_256(np.asarray(phi_w2, f))
    b2 = np.asarray(phi_b2, f)

    wr = _pack_256(np.asarray(rho_w1, f))
    # rho bias corrected for the 'stt'-shifted routes (acc is short of
    # n_shift*b2 on those feature halves).
    c = np.zeros(256, dtype=f)
    c[0:128] = sum(1 for r in PH2_ROUTE[0] if r == 'stt') * b2[0:128]
    c[128:256] = sum(1 for r in PH2_ROUTE[1] if r == 'stt') * b2[128:256]
    brv = np.asarray(rho_b1, f) + c @ np.asarray(rho_w1, f)

    wh_full = np.concatenate([np.asarray(mean_w, f), np.asarray(logstd_w, f)],
                             axis=1)                      # [256, 8]
    wh = np.concatenate([wh_full[0:128, :], wh_full[128:256, :]], axis=1)
    bh = np.concatenate([np.asarray(mean_b, f),
                         np.asarray(logstd_b, f)]).astype(f)  # [8]

    # bf16 const block: w1 | w2 | wr | wh  -> [128, 1296]
    cb = np.concatenate([w1, w2, wr, wh], axis=1).astype(NPBF16)
    # f32 const block: nb2(0:2) | b2p(2:4) | brp(4:6) | shifted clip(6:8)
    cf = np.zeros((128, 8), dtype=f)
    cf[:, 0] = -b2[0:128]
    cf[:, 1] = -b2[128:256]
    cf[:, 2] = b2[0:128]
    cf[:, 3] = b2[128:256]
    cf[:, 4] = brv[0:128]
    cf[:, 5] = brv[128:256]
    big = np.float32(3.0e38)
    hi = np.array([big] * 4 + [LOG_SIG_MAX] * 4, f) - bh
    lo = np.array([-big] * 4 + [LOG_SIG_MIN] * 4, f) - bh
    cf[0:8, 6] = hi
    cf[0:8, 7] = lo
    return dict(cb=cb, cf=cf, bh=bh)


def _pack_xt3(obs, ag, g):
    """[3, 128, B] bf16: duo d holds pair 2d at partitions 0..58 and pair
    2d+1 at partitions 64..122, each as [body;ones;A_i;A_j]."""
    B = obs.shape[0]
    xt3 = np.zeros((3, 128, B), dtype=NPBF16)
    bodyT = obs[:, 0:10].T.astype(NPBF16)
    agT = ag.T.astype(NPBF16)
    gT = g.T.astype(NPBF16)
    objT = [obs[:, 10 + 15 * o: 25 + 15 * o].T.astype(NPBF16) for o in range(3)]

    def fill_a(d, base, o):
        xt3[d, base:base + 3] = agT[3 * o:3 * o + 3]
        xt3[d, base + 3:base + 6] = gT[3 * o:3 * o + 3]
        xt3[d, base + 6 + o] = 1.0          # one-hot row
        xt3[d, base + 9:base + 24] = objT[o]

    for d in range(3):
        for half, p in ((0, 2 * d), (64, 2 * d + 1)):
            i, j = PERMS[p]
            xt3[d, half:half + 10] = bodyT
            xt3[d, half + 10] = 1.0
            fill_a(d, half + 11, i)
            fill_a(d, half + 35, j)
    return xt3


def _build_bass(bc, bt):
    nt = bc // bt
    nc = bacc.Bacc(trn_type="TRN2")

    xt3_d = nc.dram_tensor("xt3", [3, 128, bc], BF16, kind="ExternalInput")
    cb_d = nc.dram_tensor("cb", [128, 1296], BF16, kind="ExternalInput")
    cf_d = nc.dram_tensor("cf", [128, 8], F32, kind="ExternalInput")
    y_d = nc.dram_tensor("y", [8, bc], F32, kind="ExternalOutput")

    AMIN, AMAX, AADD = (mybir.AluOpType.min, mybir.AluOpType.max,
                        mybir.AluOpType.add)
    ABYP = mybir.AluOpType.bypass
    HB = bt // 2  # 512: matmul free dim / PSUM bank width

    with ExitStack() as ctx:
        tc = ctx.enter_context(tile.TileContext(nc))
        consts = ctx.enter_context(tc.tile_pool(name="consts", bufs=1))
        sbp = ctx.enter_context(tc.tile_pool(name="sbp", bufs=3))
        psp = ctx.enter_context(tc.tile_pool(name="psp", bufs=1, space="PSUM"))

        # --- warm-up: junk matmuls to lift the PE clock gate during DMA ---
        jw = consts.tile([128, 128], BF16, name="jw")
        nc.gpsimd.memset(jw, 0)
        wtile = psp.tile([64, 128], F32, tag="ps", name="wtile", bufs=4,
                         padded_shape=[128, bt])
        for _ in range(N_WARMUP_MM):
            nc.tensor.matmul(wtile, jw[:, 0:64], jw, start=True, stop=True)

        # --- input DMAs first (first tile), then consts ---
        def dma_xts(t):
            s0 = t * bt
            xts = []
            for d in range(3):
                x = sbp.tile([128, bt], BF16, tag=f"xts{d}", name=f"xts{d}",
                             bufs=2)
                nc.sync.dma_start(out=x, in_=xt3_d[d, :, s0:s0 + bt])
                xts.append(x)
            return xts

        xts = dma_xts(0)
        cbsb = consts.tile([128, 1296], BF16, name="cbsb")
        nc.sync.dma_start(out=cbsb, in_=cb_d[:, :])
        cfsb = consts.tile([128, 8], F32, name="cfsb")
        nc.sync.dma_start(out=cfsb, in_=cf_d[:, :])

        w1sb = cbsb[:, 0:256]
        w2sb = cbsb[:, 256:768]
        wrsb = cbsb[:, 768:1280]
        whsb = cbsb[:, 1280:1296]
        nb2 = cfsb[:, 0:2]
        b2p = cfsb[:, 2:4]
        brp = cfsb[:, 4:6]
        clipsb = cfsb[:, 6:8]

        def eng(name):
            return {'act': nc.scalar, 'dve': nc.vector, 'gp': nc.gpsimd}[name]

        def phi1_half(xts_d, h1duo, d, mh):
            """One duo-mh: two [128, bt] psum tiles (pair A strip 0, pair B
            strip 64, concurrent MMs), each evacuated as one FD-bt op into
            h1duo[:, mh*2048 + pair*1024 :]."""
            phs = [psp.tile([128, bt], F32, tag="ps", name="ph", bufs=4)
                   for _ in range(2)]
            for bh in range(2):
                nc.tensor.matmul(
                    phs[0][:, bh * HB:(bh + 1) * HB],
                    w1sb[0:KP, mh * 128:(mh + 1) * 128],
                    xts_d[0:KP, bh * HB:(bh + 1) * HB],
                    start=True, stop=True, tile_position=(0, 0),
                )
                nc.tensor.matmul(
                    phs[1][:, bh * HB:(bh + 1) * HB],
                    w1sb[64:64 + KP, mh * 128:(mh + 1) * 128],
                    xts_d[64:64 + KP, bh * HB:(bh + 1) * HB],
                    start=True, stop=True, tile_position=(64, 0),
                )
            ea, eb = H1_ENG[d] if mh == 0 else H1_ENG[d][::-1]
            for pair, e in ((0, ea), (1, eb)):
                dst = h1duo[:, mh * 2 * bt + pair * bt:
                            mh * 2 * bt + (pair + 1) * bt]
                src = phs[pair]
                if e == 'act':
                    nc.scalar.activation(dst, src, RELU)
                else:
                    nc.vector.tensor_scalar_max(dst, src, 0.0)

        def phi2_pair(h1duo, pairoff, p, st):
            """phi2 MMs + fused relu/pool evacuation for one pair.
            h1duo layout: [128, 4096] = (k=0: A-b0 A-b1 B-b0 B-b1 | k=1: ...)
            """
            for mh in range(2):
                ph2 = psp.tile([128, bt], F32, tag="ps", name="ph2", bufs=4)
                for bh in range(2):
                    for k in range(2):
                        nc.tensor.matmul(
                            ph2[:, bh * HB:(bh + 1) * HB],
                            w2sb[:, (2 * k + mh) * 128:(2 * k + mh + 1) * 128],
                            h1duo[:, k * 2 * bt + pairoff * bt + bh * HB:
                                  k * 2 * bt + pairoff * bt + (bh + 1) * HB],
                            start=(k == 0), stop=(k == 1),
                        )
                route = PH2_ROUTE[mh][p]
                acc_mh = st["acc"][:, mh * bt:(mh + 1) * bt]
                if route == 'stt':
                    if st["init"][mh]:
                        nc.vector.tensor_scalar(
                            acc_mh, ph2, nb2[:, mh:mh + 1], 0.0,
                            op0=AMAX, op1=ABYP)
                    else:
                        nc.vector.scalar_tensor_tensor(
                            acc_mh, ph2, nb2[:, mh:mh + 1], acc_mh,
                            op0=AMAX, op1=AADD)
                else:  # 'act' route: true relu(z + b2)
                    if st["init"][mh]:
                        nc.scalar.activation(acc_mh, ph2, RELU,
                                             bias=b2p[:, mh:mh + 1])
                    else:
                        r = sbp.tile([128, bt], BF16, tag="rtmp", name="r",
                                     bufs=4)
                        nc.scalar.activation(r, ph2, RELU,
                                             bias=b2p[:, mh:mh + 1])
                        # pair-tree on GPSIMD: shallower dependency chain
                        # than a serial acc += r for every pair. The last
                        # tile drains the pipeline, so its adds go on the
                        # (then idle, much faster) DVE instead.
                        adder = nc.vector if st["last"] else nc.gpsimd
                        if st["pend"][mh] is None:
                            st["pend"][mh] = r
                        else:
                            s = sbp.tile([128, bt], BF16, tag="rtmp",
                                         name="s", bufs=4)
                            adder.tensor_add(s, st["pend"][mh], r)
                            adder.tensor_add(acc_mh, acc_mh, s)
                            st["pend"][mh] = None
                st["init"][mh] = False
            if p == 5:  # flush leftover tree terms (DVE: shortens the
                # acc critical path that gates the rho matmuls)
                adder = nc.vector
                for mh in range(2):
                    if st["pend"][mh] is not None:
                        acc_mh = st["acc"][:, mh * bt:(mh + 1) * bt]
                        adder.tensor_add(acc_mh, acc_mh, st["pend"][mh])
                        st["pend"][mh] = None

        def start_tile_state(t):
            acc = sbp.tile([128, 2 * bt], BF16, tag="acc", name="acc", bufs=3)
            return {"t": t, "acc": acc, "init": [True, True],
                    "pend": [None, None], "last": t == nt - 1}

        def finisher(st):
            """rho + heads + clip + store, as 3 stages interleaved with the
            next tile's duos."""
            t = st["t"]
            s0 = t * bt
            acc = st["acc"]
            fstate = {}

            def stage_a():  # rho matmuls + rho evac
                prs = []
                for m in range(2):
                    pr = psp.tile([128, bt], F32, tag="ps", name="pr", bufs=4)
                    for bh in range(2):
                        for k in range(2):
                            nc.tensor.matmul(
                                pr[:, bh * HB:(bh + 1) * HB],
                                wrsb[:, (2 * k + m) * 128:
                                     (2 * k + m + 1) * 128],
                                acc[:, k * bt + bh * HB:k * bt + (bh + 1) * HB],
                                start=(k == 0), stop=(k == 1),
                            )
                    prs.append(pr)
                xs = sbp.tile([128, 2 * bt], BF16, tag="xs", name="xs", bufs=2)
                for m in range(2):
                    nc.scalar.activation(
                        xs[:, m * bt:(m + 1) * bt], prs[m],
                        RELU, bias=brp[:, m:m + 1])
                fstate["xs"] = xs

            def stage_b():  # head matmuls
                xs = fstate["xs"]
                py = psp.tile([8, bt], F32, tag="ps", name="py", bufs=4)
                for bh in range(2):
                    for k in range(2):
                        nc.tensor.matmul(
                            py[:, bh * HB:(bh + 1) * HB],
                            whsb[:, k * 8:(k + 1) * 8],
                            xs[:, k * bt + bh * HB:k * bt + (bh + 1) * HB],
                            start=(k == 0), stop=(k == 1),
                        )
                fstate["py"] = py

            def stage_c():  # shifted clip + store (bias added on host)
                py = fstate["py"]
                ysb = sbp.tile([8, bt], F32, tag="ysb", name="ysb")
                nc.vector.tensor_scalar(
                    ysb, py, clipsb[0:8, 0:1], clipsb[0:8, 1:2],
                    op0=AMIN, op1=AMAX,
                )
                nc.sync.dma_start(out=y_d[:, s0:s0 + bt], in_=ysb)

            return [stage_a, stage_b, stage_c]

        # ---------- main pipeline ----------
        # phi2 lags phi1 by TWO duos (slack for the h1 evacuations), the
        # finisher lags a further tile (slack for the GPSIMD pool-add tree).
        from collections import deque
        prevq = deque()        # (h1duo, state, duo_idx), oldest first
        finq = deque()         # pending finisher stage closures
        xts_next = None
        def duo_slot(xts_d, d):
            """One pipeline slot: run a pending finisher stage (enqueued at
            least one slot ago), this duo's phi1, and the 2-slot-lagged
            duo's phi2."""
            if finq:
                finq.popleft()()
            lag = prevq[0] if (len(prevq) >= 2 or
                               (xts_d is None and prevq)) else None
            h1duo = None
            if xts_d is not None:
                h1duo = sbp.tile([128, 2 * 2 * bt], BF16, tag="h1",
                                 name="h1duo", bufs=3)
                phi1_half(xts_d, h1duo, d, 0)
            if lag is not None:
                (ph1s, pst, pd) = lag
                phi2_pair(ph1s, 0, 2 * pd, pst)
            if xts_d is not None:
                phi1_half(xts_d, h1duo, d, 1)
            if lag is not None:
                phi2_pair(ph1s, 1, 2 * pd + 1, pst)
                prevq.popleft()
                if pd == 2:
                    finq.extend(finisher(pst))
            if h1duo is not None:
                prevq.append((h1duo, st, d))

        for t in range(nt):
            st = start_tile_state(t)
            if t + 1 < nt:
                xts_next = dma_xts(t + 1)
            for d in range(3):
                duo_slot(xts[d], d)
            if t + 1 < nt:
                xts = xts_next
        # drain: no new phi1 work, keep running lagged phi2 + finishers
        while prevq or finq:
            duo_slot(None, 0)

    return nc


def _get_nc(bc, bt):
    key = (bc, bt)
    if key not in _CACHE:
        nc = _build_bass(bc, bt)
        nc.finalize()
        _CACHE[key] = nc
    return _CACHE[key]


def kernel(obs, ag, g, phi_w1, phi_b1, phi_w2, phi_b2,
           rho_w1, rho_b1, mean_w, mean_b, logstd_w, logstd_b):
    obs = np.asarray(obs, np.float32)
    ag = np.asarray(ag, np.float32)
    g = np.asarray(g, np.float32)
    B = obs.shape[0]
    assert B == B_FULL, f"kernel hardcoded for B={B_FULL}, got {B}"

    packed = _pack_weights(phi_w1, phi_b1, phi_w2, phi_b2, rho_w1, rho_b1,
                           mean_w, mean_b, logstd_w, logstd_b)
    bh = packed.pop("bh")
    xt3 = _pack_xt3(obs, ag, g)

    nc = _get_nc(BC, BT)
    in_maps = []
    for c in range(N_CORES):
        m = dict(packed)
        m["xt3"] = np.ascontiguousarray(xt3[:, :, c * BC:(c + 1) * BC])
        in_maps.append(m)

    import os
    trace = bool(os.environ.get("KERNEL_TRACE"))
    res = run_bass_kernel_spmd(nc, in_maps, core_ids=list(range(N_CORES)),
                               trace=trace)
    global _last_results
    _last_results = res

    y = np.concatenate([res.results[c]["y"] for c in range(N_CORES)], axis=1)
    out = np.ascontiguousarray(y.T) + bh[None, :]  # host-side head bias
    mean = out[:, 0:4].copy()
    logstd = out[:, 4:8].copy()
    return mean, logstd


_last_results = None
